# revision 17
# baseline (speedup 1.0000x reference)
"""Trainium2 Bass kernel for the ConvolutionalCapsule module.

Sharding: data-parallel over (batch, H-half): core k handles b = k//2,
output rows h in [6*(k%2), 6*(k%2)+6), i.e. 72 spatial positions per core.
Weights are replicated. All FLOPs run on-device; the host only does
layout/gather of inputs (patch extraction + weight transposes).

Device algorithm per core (pos = 72 positions), with the (c,i) axis packed
densely into 18 chunks of 128 rows:
  - 16 "full" chunks: chunk u = i*2+cb holds c in [cb*128,(cb+1)*128) x {i}
  - 2 "packed" chunks: chunk 16+ip holds rows (isub*32 + c-256) for
    c in [256,288), i = 4*ip+isub  (the 32-wide tail c-block x 4 i's)

  iter0:  out0 = squash(mean_c preds) via K=(c,i)=2304 matmuls (stage B).
  iter1:  V[f,(c,i)] = sum_o W * out0   (per-f K=16 matmuls -> fp32 PSUM,
                                         fp8 weights, 4 row-bands in flight)
          VP = V * P                    (DVE direct from PSUM or ScalarE
                                         evacuation + DVE 2x mult)
          agr = sum_i VP                (identity matmuls N=144 for full
                                         chunks + block-ones matmuls N=72
                                         for packed chunks, PSUM accum)
          e = exp(agr) on ScalarE straight out of PSUM (one per f-pair),
          Z = sum_f e via identity-matmul accumulation,
          pp = P / Z, S_f = e_f * pp,
          centroids = sum_{(c,i)} S_f * W  (per-f K=128 chunk matmuls)
          out = squash(centroids)
The packed-chunk agr rows are replicated across the four 32-row groups
(block-ones weight), so exp/Z/pp/S stay purely elementwise.
"""
import numpy as np

KH = KW = 3
B, H, WD, FIN, DIN = 4, 14, 14, 32, 8
F, C, DO, DI = 32, 288, 16, 8
NPOS = 72
NCH = 18            # dense chunks
NFULL = 16
EPS = 1e-7

W8 = True   # fp8 weights for the V-stage (w_vt)

_CACHE: dict = {}


def _chunks_p():
    """Per chunk: (c_idx[128], i_idx[128]) — row r holds (c_idx[r], i_idx[r])."""
    chunks = []
    for i in range(DI):
        for cb in range(2):
            c = np.arange(cb * 128, (cb + 1) * 128)
            chunks.append((c, np.full(128, i)))
    r = np.arange(128)
    for ip in range(2):
        chunks.append((256 + r % 32, 4 * ip + r // 32))
    return chunks


def _host_weights(Wm):
    """Wm: [F, C, DO, DI] float32 -> device weight layouts."""
    chunks = _chunks_p()
    w_r = np.empty((NCH, 128, F * DO), np.float16)
    w_vt = np.zeros((4, 32, 8, NCH, 128), np.float32)
    for u, (c_idx, i_idx) in enumerate(chunks):
        # [128, F, DO] with row r = (c_idx[r], i_idx[r])
        blk = Wm[:, c_idx, :, i_idx]
        w_r[u] = blk.reshape(128, F * DO)
        for f in range(F):
            g, j = divmod(f, 4)
            w_vt[j, :DO, g, u, :] = blk[:, f, :].T
    w_r = w_r.transpose(1, 0, 2).reshape(128, NCH * F * DO).copy()
    w_vt = w_vt.reshape(128, 8 * NCH * 128)
    if W8:
        import ml_dtypes
        w_vt = w_vt.astype(ml_dtypes.float8_e4m3fn)
    else:
        w_vt = w_vt.astype(np.float16)
    return w_r, w_vt


def _host_patches(x, k):
    """Patch tensor for core k in dense chunk layout: [128, 18*72] fp16."""
    b, hh = divmod(k, 2)
    h0 = 6 * hh
    P = np.empty((6, 12, KH, KW, FIN, DIN), np.float32)
    for kh in range(KH):
        for kw in range(KW):
            for h in range(6):
                P[h, :, kh, kw] = x[b, h0 + h + kh, kw:kw + 12]
    P = P.reshape(NPOS, C, DIN)
    p_ct = np.empty((NCH, 128, NPOS), np.float16)
    for u, (c_idx, i_idx) in enumerate(_chunks_p()):
        p_ct[u] = P[:, c_idx, i_idx].T
    return p_ct.transpose(1, 0, 2).reshape(128, NCH * NPOS).copy()


def _build():
    import concourse.bass as bass
    import concourse.bacc as bacc
    import concourse.mybir as mybir
    import concourse.tile as tile

    F16, F32 = mybir.dt.float16, mybir.dt.float32
    W_DT = mybir.dt.float8e4 if W8 else F16
    AX = mybir.AxisListType
    AF = mybir.ActivationFunctionType

    nc = bacc.Bacc(None, target_bir_lowering=False, debug=False)

    p_ct_d = nc.dram_tensor("p_ct", [128, NCH * NPOS], F16, kind="ExternalInput")
    w_r_d = nc.dram_tensor("w_r", [128, NCH * F * DO], F16, kind="ExternalInput")
    w_vt_d = nc.dram_tensor("w_vt", [128, 8 * NCH * 128], W_DT, kind="ExternalInput")
    eye72_d = nc.dram_tensor("eye72", [NPOS, NPOS], F32, kind="ExternalInput")
    eye128h_d = nc.dram_tensor("eye128h", [128, 128], F16, kind="ExternalInput")
    bones_d = nc.dram_tensor("bones", [128, 128], F16, kind="ExternalInput")
    eye128f_d = nc.dram_tensor("eye128f", [128, 128], F32, kind="ExternalInput")
    y_d = nc.dram_tensor("y", [NPOS, F * DO], F32, kind="ExternalOutput")

    NCC = NCH * NPOS          # 1296 cols of P/VP/S
    FSL = 2 * NPOS            # 144 = full-chunk slice of agr (cb,pos)
    PSL = NPOS                # 72  = packed-chunk (replicated) slice
    ASL = FSL + PSL           # 216 = per-f agr/e slice

    with tile.TileContext(nc) as tc:
        with (
            tc.tile_pool(name="const", bufs=1) as const,
            tc.tile_pool(name="work", bufs=1) as work,
            tc.tile_pool(name="vpr", bufs=6) as vpr,
            tc.tile_pool(name="sr", bufs=6) as sr,
            tc.tile_pool(name="vqp", bufs=6, space=bass.MemorySpace.PSUM) as vqp,
            tc.tile_pool(name="ps1", bufs=2, space=bass.MemorySpace.PSUM) as ps1,
        ):
            # ---------------- loads ----------------
            p_ct = const.tile([128, NCC], F16, tag="p_ct")
            nc.sync.dma_start(p_ct[:], p_ct_d[:])
            eye72 = const.tile([NPOS, NPOS], F32, tag="eye72")
            nc.sync.dma_start(eye72[:], eye72_d[:])
            eye128h = const.tile([128, 128], F16, tag="eye128h")
            nc.sync.dma_start(eye128h[:], eye128h_d[:])
            bones = const.tile([128, 128], F16, tag="bones")
            nc.sync.dma_start(bones[:], bones_d[:])
            eye128f = const.tile([128, 128], F32, tag="eye128f")
            nc.sync.dma_start(eye128f[:], eye128f_d[:])
            w_r = const.tile([128, NCH * F * DO], F16, tag="w_r")
            WRC = F * DO
            for h in range(3):
                nc.sync.dma_start(
                    w_r[:, h * 6 * WRC:(h + 1) * 6 * WRC],
                    w_r_d[:, h * 6 * WRC:(h + 1) * 6 * WRC],
                )
            w_vt = const.tile([128, 8 * NCH * 128], W_DT, tag="w_vt")
            GSZ = NCH * 128
            for g in range(8):
                nc.sync.dma_start(
                    w_vt[:, g * GSZ:(g + 1) * GSZ],
                    w_vt_d[:, g * GSZ:(g + 1) * GSZ],
                )

            def squash(src_ap, dst_ap, pre_scale, tag):
                """dst = squash(src * pre_scale) ; src/dst free = (f,o)=512."""
                s = work.tile([NPOS, F * DO], F32, tag=f"{tag}_s")
                nc.scalar.activation(s[:], src_ap, AF.Copy, scale=pre_scale)
                sq = work.tile([NPOS, F * DO], F32, tag=f"{tag}_sq")
                nc.scalar.activation(sq[:], s[:], AF.Square)
                sn = work.tile([NPOS, F], F32, tag=f"{tag}_sn")
                nc.vector.reduce_sum(
                    sn[:], sq[:].rearrange("p (f o) -> p f o", o=DO), axis=AX.X
                )
                t1 = work.tile([NPOS, F], F32, tag=f"{tag}_t1")
                nc.vector.tensor_scalar_add(t1[:], sn[:], 1.0)
                r1 = work.tile([NPOS, F], F32, tag=f"{tag}_r1")
                nc.vector.reciprocal(r1[:], t1[:])
                se = work.tile([NPOS, F], F32, tag=f"{tag}_se")
                nc.vector.tensor_scalar_add(se[:], sn[:], EPS)
                r2 = work.tile([NPOS, F], F32, tag=f"{tag}_r2")
                nc.scalar.activation(r2[:], se[:], AF.Sqrt)
                r3 = work.tile([NPOS, F], F32, tag=f"{tag}_r3")
                nc.vector.reciprocal(r3[:], r2[:])
                sc = work.tile([NPOS, F], F32, tag=f"{tag}_sc")
                nc.vector.tensor_mul(sc[:], sn[:], r1[:])
                sc2 = work.tile([NPOS, F], F32, tag=f"{tag}_sc2")
                nc.vector.tensor_mul(sc2[:], sc[:], r3[:])
                bc = sc2[:].unsqueeze(2).broadcast_to((NPOS, F, DO))
                nc.vector.tensor_mul(
                    dst_ap, s[:].rearrange("p (f o) -> p f o", o=DO), bc
                )

            # ---------------- stage B: out0 ----------------
            o0p = ps1.tile([NPOS, F * DO], F32, tag="ps1")
            for t in range(NCH):
                nc.tensor.matmul(
                    o0p[:],
                    p_ct[:, t * NPOS:(t + 1) * NPOS],
                    w_r[:, t * F * DO:(t + 1) * F * DO],
                    start=(t == 0),
                    stop=(t == NCH - 1),
                )
            out0_pad = work.tile([NPOS, F * 32], F32, tag="out0_pad")
            nc.vector.memset(out0_pad[:], 0.0)
            squash(
                o0p[:],
                out0_pad[:].rearrange("p (f s) -> p f s", s=32)[:, :, 0:DO],
                1.0 / F,
                "sq1",
            )
            # transposes -> out0T [128 = (j,o-slot), 8*72]
            out0T = work.tile([128, 8 * NPOS], F16, tag="out0T")
            for g in range(8):
                tp = ps1.tile([128, 128], F32, tag="ps1")
                nc.tensor.transpose(
                    tp[:, 0:NPOS], out0_pad[:, g * 128:(g + 1) * 128], eye72[:]
                )
                nc.scalar.copy(out0T[:, g * NPOS:(g + 1) * NPOS], tp[:, 0:NPOS])

            # ---------------- stage D: V, VP, agreement, exp ----------------
            # e[(c-rows), (f, 144 full | 72 packed)] = exp(agr)
            e = work.tile([128, F * ASL], F16, tag="e")
            TCH = NCH // 3      # 6 chunks per third
            TSZ = TCH * NPOS    # 432 cols = 1728 B = one fp32 PSUM bank
            def emit_agr(g, VPs):
                for jp in range(2):
                    # agr for a j-pair in one 1-bank psum tile; single exp.
                    # Per f one accumulation group covers both the full-chunk
                    # slice (identity weight, N=144) and the packed slice
                    # (block-ones weight -> 4-row-group replicated sum, N=72).
                    agp = ps1.tile([128, 2 * ASL], F32, tag="ps1",
                                   name=f"agp{g}_{jp}")
                    for jj in range(2):
                        j = 2 * jp + jj
                        base = jj * ASL
                        for i in range(DI):
                            nc.tensor.matmul(
                                agp[:, base:base + FSL],
                                eye128h[:],
                                VPs[j][:, i * FSL:(i + 1) * FSL],
                                start=(i == 0),
                                stop=False,
                            )
                        for ip in range(2):
                            nc.tensor.matmul(
                                agp[:, base + FSL:base + ASL],
                                bones[:],
                                VPs[j][:, NFULL * NPOS + ip * NPOS:
                                        NFULL * NPOS + (ip + 1) * NPOS],
                                start=False,
                                stop=(ip == 1),
                            )
                    f0 = 4 * g + 2 * jp
                    nc.scalar.activation(
                        e[:, f0 * ASL:(f0 + 2) * ASL], agp[:], AF.Exp
                    )

            prev = None
            for g in range(8):
                # 4 f's (one per row-band j) in flight: consecutive matmuls
                # target different 32-row groups so LDWEIGHTS can overlap.
                VPs = [vpr.tile([128, NCC], F16, tag="VP", name=f"VP{g}_{jj}")
                       for jj in range(4)]
                for q in range(3):
                    vqs = [vqp.tile([128, TSZ], F32, tag="vq",
                                    name=f"vq{g}_{q}_{jj}")
                           for jj in range(4)]
                    for u in range(TCH):
                        t = q * TCH + u
                        for j in range(4):
                            nc.tensor.matmul(
                                vqs[j][:, u * NPOS:(u + 1) * NPOS],
                                w_vt[32 * j:32 * (j + 1),
                                     (g * NCH + t) * 128:
                                     (g * NCH + t + 1) * 128],
                                out0T[32 * j:32 * (j + 1),
                                      g * NPOS:(g + 1) * NPOS],
                                start=True,
                                stop=True,
                                tile_position=(32 * j, 0),
                            )
                    for j in range(4):
                        if j % 2 == 0:
                            # VP = V * P straight from PSUM on DVE (1x)
                            nc.vector.tensor_mul(
                                VPs[j][:, q * TSZ:(q + 1) * TSZ],
                                vqs[j][:],
                                p_ct[:, q * TSZ:(q + 1) * TSZ],
                            )
                        else:
                            # ScalarE evacuation -> fp16 SBUF, DVE 2x mult
                            vs = sr.tile([128, TSZ], F16, tag="vs")
                            nc.scalar.copy(vs[:], vqs[j][:])
                            nc.vector.tensor_mul(
                                VPs[j][:, q * TSZ:(q + 1) * TSZ],
                                vs[:],
                                p_ct[:, q * TSZ:(q + 1) * TSZ],
                            )
                if prev is not None:
                    emit_agr(g - 1, prev)
                prev = VPs
            emit_agr(7, prev)

            # ---------------- softmax pieces ----------------
            # Zf = sum_f e  (identity-matmul accumulation)
            Zf = ps1.tile([128, ASL], F32, tag="ps1")
            for f in range(F):
                nc.tensor.matmul(
                    Zf[:],
                    eye128h[:],
                    e[:, f * ASL:(f + 1) * ASL],
                    start=(f == 0),
                    stop=(f == F - 1),
                )
            Zr = work.tile([128, ASL], F16, tag="Zr")
            with nc.allow_low_precision(reason="1/Z in [1e-2,1e-1]; fp16 ok"):
                nc.vector.reciprocal(Zr[:], Zf[:])
            pp = work.tile([128, NCC], F16, tag="pp")
            nc.vector.tensor_mul(
                pp[:, 0:NFULL * NPOS]
                .rearrange("p (i cb n) -> p i cb n", i=DI, cb=2),
                p_ct[:, 0:NFULL * NPOS]
                .rearrange("p (i cb n) -> p i cb n", i=DI, cb=2),
                Zr[:, 0:FSL].rearrange("p (cb n) -> p cb n", cb=2)
                .unsqueeze(1)
                .broadcast_to((128, DI, 2, NPOS)),
            )
            nc.vector.tensor_mul(
                pp[:, NFULL * NPOS:NCC].rearrange("p (ip n) -> p ip n", ip=2),
                p_ct[:, NFULL * NPOS:NCC].rearrange("p (ip n) -> p ip n", ip=2),
                Zr[:, FSL:ASL].unsqueeze(1).broadcast_to((128, 2, NPOS)),
            )

            # ---------------- stage S + centroids ----------------
            cen_sb = work.tile([128, 8 * NPOS], F32, tag="cen_sb")
            for g in range(8):
                cg = ps1.tile([128, NPOS], F32, tag="ps1")
                for j in range(4):
                    f = 4 * g + j
                    S = sr.tile([128, NCC], F16, tag="S")
                    nc.vector.tensor_mul(
                        S[:, 0:NFULL * NPOS]
                        .rearrange("p (i cb n) -> p i cb n", i=DI, cb=2),
                        pp[:, 0:NFULL * NPOS]
                        .rearrange("p (i cb n) -> p i cb n", i=DI, cb=2),
                        e[:, f * ASL:f * ASL + FSL]
                        .rearrange("p (cb n) -> p cb n", cb=2)
                        .unsqueeze(1)
                        .broadcast_to((128, DI, 2, NPOS)),
                    )
                    nc.vector.tensor_mul(
                        S[:, NFULL * NPOS:NCC]
                        .rearrange("p (ip n) -> p ip n", ip=2),
                        pp[:, NFULL * NPOS:NCC]
                        .rearrange("p (ip n) -> p ip n", ip=2),
                        e[:, f * ASL + FSL:(f + 1) * ASL]
                        .unsqueeze(1).broadcast_to((128, 2, NPOS)),
                    )
                    for t in range(NCH):
                        nc.tensor.matmul(
                            cg[32 * j:32 * j + DO, :],
                            w_r[:, t * F * DO + f * DO:t * F * DO + (f + 1) * DO],
                            S[:, t * NPOS:(t + 1) * NPOS],
                            start=(t == 0),
                            stop=(t == NCH - 1),
                            tile_position=(0, 32 * j),
                        )
                nc.scalar.copy(cen_sb[:, g * NPOS:(g + 1) * NPOS], cg[:])

            # ---------------- squash2 + output ----------------
            opre = work.tile([NPOS, 8 * 128], F32, tag="opre")
            for g in range(8):
                tp2 = ps1.tile([128, 128], F32, tag="ps1")
                nc.tensor.transpose(
                    tp2[0:NPOS, :], cen_sb[:, g * NPOS:(g + 1) * NPOS], eye128f[:]
                )
                nc.scalar.copy(opre[:, g * 128:(g + 1) * 128], tp2[0:NPOS, :])
            gat = work.tile([NPOS, F * DO], F32, tag="gat")
            nc.vector.tensor_copy(
                gat[:].rearrange("p (g j o) -> p g j o", g=8, j=4),
                opre[:].rearrange("p (g j s) -> p g j s", g=8, j=4)[:, :, :, 0:DO],
            )
            y_sb = work.tile([NPOS, F * DO], F32, tag="y_sb")
            squash(gat[:], y_sb[:].rearrange("p (f o) -> p f o", o=DO), 1.0, "sq2")
            nc.sync.dma_start(y_d[:], y_sb[:])

    nc.compile()
    return nc


def _get_program():
    if "nc" not in _CACHE:
        _CACHE["nc"] = _build()
    return _CACHE["nc"]


def _make_in_maps(x, W):
    x = np.asarray(x, np.float32)
    Wm = np.asarray(W, np.float32)[0, 0, 0]
    w_r, w_vt = _host_weights(Wm)
    eye72 = np.eye(NPOS, dtype=np.float32)
    eye128h = np.eye(128, dtype=np.float16)
    bones = np.tile(np.eye(32, dtype=np.float16), (4, 4))
    eye128f = np.eye(128, dtype=np.float32)
    in_maps = []
    for k in range(8):
        in_maps.append({
            "p_ct": _host_patches(x, k),
            "w_r": w_r,
            "w_vt": w_vt,
            "eye72": eye72,
            "eye128h": eye128h,
            "bones": bones,
            "eye128f": eye128f,
        })
    return in_maps


def kernel(x, W):
    from concourse.bass_utils import run_bass_kernel_spmd

    nc = _get_program()
    in_maps = _make_in_maps(x, W)
    res = run_bass_kernel_spmd(nc, in_maps, list(range(8)))
    Ho, Wo = H - KH + 1, WD - KW + 1
    y = np.empty((B, Ho, Wo, F, DO), np.float32)
    for k in range(8):
        b, hh = divmod(k, 2)
        y[b, 6 * hh:6 * hh + 6] = res.results[k]["y"].reshape(6, Wo, F, DO)
    return y


# revision 18
# speedup vs baseline: 1.0967x; 1.0967x over previous
"""Trainium2 Bass kernel for the ConvolutionalCapsule module.

Sharding: data-parallel over (batch, H-half): core k handles b = k//2,
output rows h in [6*(k%2), 6*(k%2)+6), i.e. 72 spatial positions per core.
Weights are replicated. All FLOPs run on-device; the host only does
layout/gather of inputs (patch extraction + weight transposes).

Device algorithm per core (pos = 72 positions), with the (c,i) axis packed
densely into 18 chunks of 128 rows:
  - 16 "full" chunks: chunk u = i*2+cb holds c in [cb*128,(cb+1)*128) x {i}
  - 2 "packed" chunks: chunk 16+ip holds rows (isub*32 + c-256) for
    c in [256,288), i = 4*ip+isub  (the 32-wide tail c-block x 4 i's)

  iter0:  out0 = squash(mean_c preds) via K=(c,i)=2304 matmuls (stage B).
  iter1:  V[f,(c,i)] = sum_o W * out0   (per-f K=16 matmuls -> fp32 PSUM,
                                         fp8 weights, 4 row-bands in flight)
          VP = V * P                    (DVE direct from PSUM or ScalarE
                                         evacuation + DVE 2x mult)
          agr = sum_i VP                (identity matmuls N=144 for full
                                         chunks + block-ones matmuls N=72
                                         for packed chunks, PSUM accum)
          e = exp(agr) on ScalarE straight out of PSUM (one per f-pair),
          Z = sum_f e via identity-matmul accumulation,
          pp = P / Z, S_f = e_f * pp,
          centroids = sum_{(c,i)} S_f * W  (per-f K=128 chunk matmuls)
          out = squash(centroids)
The packed-chunk agr rows are replicated across the four 32-row groups
(block-ones weight), so exp/Z/pp/S stay purely elementwise.
"""
import numpy as np

KH = KW = 3
B, H, WD, FIN, DIN = 4, 14, 14, 32, 8
F, C, DO, DI = 32, 288, 16, 8
NPOS = 72
NCH = 18            # dense chunks
NFULL = 16
EPS = 1e-7

W8 = True   # fp8 weights for the V-stage (w_vt)

_CACHE: dict = {}


def _chunks_p():
    """Per chunk: (c_idx[128], i_idx[128]) — row r holds (c_idx[r], i_idx[r])."""
    chunks = []
    for i in range(DI):
        for cb in range(2):
            c = np.arange(cb * 128, (cb + 1) * 128)
            chunks.append((c, np.full(128, i)))
    r = np.arange(128)
    for ip in range(2):
        chunks.append((256 + r % 32, 4 * ip + r // 32))
    return chunks


def _host_weights(Wm):
    """Wm: [F, C, DO, DI] float32 -> device weight layouts."""
    chunks = _chunks_p()
    w_r = np.empty((NCH, 128, F * DO), np.float16)
    w_vt = np.zeros((4, 32, 8, NCH, 128), np.float32)
    for u, (c_idx, i_idx) in enumerate(chunks):
        # [128, F, DO] with row r = (c_idx[r], i_idx[r])
        blk = Wm[:, c_idx, :, i_idx]
        w_r[u] = blk.reshape(128, F * DO)
        for f in range(F):
            g, j = divmod(f, 4)
            w_vt[j, :DO, g, u, :] = blk[:, f, :].T
    w_r = w_r.transpose(1, 0, 2).reshape(128, NCH * F * DO).copy()
    w_vt = w_vt.reshape(128, 8 * NCH * 128)
    if W8:
        import ml_dtypes
        w_vt = w_vt.astype(ml_dtypes.float8_e4m3fn)
    else:
        w_vt = w_vt.astype(np.float16)
    return w_r, w_vt


def _host_patches(x, k):
    """Patch tensor for core k in dense chunk layout: [128, 18*72] fp16."""
    b, hh = divmod(k, 2)
    h0 = 6 * hh
    P = np.empty((6, 12, KH, KW, FIN, DIN), np.float32)
    for kh in range(KH):
        for kw in range(KW):
            for h in range(6):
                P[h, :, kh, kw] = x[b, h0 + h + kh, kw:kw + 12]
    P = P.reshape(NPOS, C, DIN)
    p_ct = np.empty((NCH, 128, NPOS), np.float16)
    for u, (c_idx, i_idx) in enumerate(_chunks_p()):
        p_ct[u] = P[:, c_idx, i_idx].T
    return p_ct.transpose(1, 0, 2).reshape(128, NCH * NPOS).copy()


def _build():
    import concourse.bass as bass
    import concourse.bacc as bacc
    import concourse.mybir as mybir
    import concourse.tile as tile

    F16, F32 = mybir.dt.float16, mybir.dt.float32
    W_DT = mybir.dt.float8e4 if W8 else F16
    AX = mybir.AxisListType
    AF = mybir.ActivationFunctionType

    nc = bacc.Bacc(None, target_bir_lowering=False, debug=False)

    p_ct_d = nc.dram_tensor("p_ct", [128, NCH * NPOS], F16, kind="ExternalInput")
    w_r_d = nc.dram_tensor("w_r", [128, NCH * F * DO], F16, kind="ExternalInput")
    w_vt_d = nc.dram_tensor("w_vt", [128, 8 * NCH * 128], W_DT, kind="ExternalInput")
    eye72_d = nc.dram_tensor("eye72", [NPOS, NPOS], F32, kind="ExternalInput")
    eye128h_d = nc.dram_tensor("eye128h", [128, 128], F16, kind="ExternalInput")
    bones_d = nc.dram_tensor("bones", [128, 128], F16, kind="ExternalInput")
    eye128f_d = nc.dram_tensor("eye128f", [128, 128], F32, kind="ExternalInput")
    y_d = nc.dram_tensor("y", [NPOS, F * DO], F32, kind="ExternalOutput")

    NCC = NCH * NPOS          # 1296 cols of P/VP/S
    FSL = 2 * NPOS            # 144 = full-chunk slice of agr (cb,pos)
    PSL = NPOS                # 72  = packed-chunk (replicated) slice
    ASL = FSL + PSL           # 216 = per-f agr/e slice

    with tile.TileContext(nc) as tc:
        with (
            tc.tile_pool(name="const", bufs=1) as const,
            tc.tile_pool(name="work", bufs=1) as work,
            tc.tile_pool(name="vpr", bufs=6) as vpr,
            tc.tile_pool(name="sr", bufs=6) as sr,
            tc.tile_pool(name="vqp", bufs=6, space=bass.MemorySpace.PSUM) as vqp,
            tc.tile_pool(name="ps1", bufs=2, space=bass.MemorySpace.PSUM) as ps1,
        ):
            # ---------------- loads ----------------
            p_ct = const.tile([128, NCC], F16, tag="p_ct")
            nc.sync.dma_start(p_ct[:], p_ct_d[:])
            eye72 = const.tile([NPOS, NPOS], F32, tag="eye72")
            nc.sync.dma_start(eye72[:], eye72_d[:])
            eye128h = const.tile([128, 128], F16, tag="eye128h")
            nc.sync.dma_start(eye128h[:], eye128h_d[:])
            bones = const.tile([128, 128], F16, tag="bones")
            nc.sync.dma_start(bones[:], bones_d[:])
            eye128f = const.tile([128, 128], F32, tag="eye128f")
            nc.sync.dma_start(eye128f[:], eye128f_d[:])
            w_r = const.tile([128, NCH * F * DO], F16, tag="w_r")
            WRC = F * DO
            for h in range(3):
                nc.sync.dma_start(
                    w_r[:, h * 6 * WRC:(h + 1) * 6 * WRC],
                    w_r_d[:, h * 6 * WRC:(h + 1) * 6 * WRC],
                )
            w_vt = const.tile([128, 8 * NCH * 128], W_DT, tag="w_vt")
            GSZ = NCH * 128
            for g in range(8):
                nc.sync.dma_start(
                    w_vt[:, g * GSZ:(g + 1) * GSZ],
                    w_vt_d[:, g * GSZ:(g + 1) * GSZ],
                )

            def squash(src_ap, dst_ap, pre_scale, tag):
                """dst = squash(src * pre_scale) ; src/dst free = (f,o)=512."""
                s = work.tile([NPOS, F * DO], F32, tag=f"{tag}_s")
                nc.scalar.activation(s[:], src_ap, AF.Copy, scale=pre_scale)
                sq = work.tile([NPOS, F * DO], F32, tag=f"{tag}_sq")
                nc.scalar.activation(sq[:], s[:], AF.Square)
                sn = work.tile([NPOS, F], F32, tag=f"{tag}_sn")
                nc.vector.reduce_sum(
                    sn[:], sq[:].rearrange("p (f o) -> p f o", o=DO), axis=AX.X
                )
                t1 = work.tile([NPOS, F], F32, tag=f"{tag}_t1")
                nc.vector.tensor_scalar_add(t1[:], sn[:], 1.0)
                r1 = work.tile([NPOS, F], F32, tag=f"{tag}_r1")
                nc.vector.reciprocal(r1[:], t1[:])
                se = work.tile([NPOS, F], F32, tag=f"{tag}_se")
                nc.vector.tensor_scalar_add(se[:], sn[:], EPS)
                r2 = work.tile([NPOS, F], F32, tag=f"{tag}_r2")
                nc.scalar.activation(r2[:], se[:], AF.Sqrt)
                r3 = work.tile([NPOS, F], F32, tag=f"{tag}_r3")
                nc.vector.reciprocal(r3[:], r2[:])
                sc = work.tile([NPOS, F], F32, tag=f"{tag}_sc")
                nc.vector.tensor_mul(sc[:], sn[:], r1[:])
                sc2 = work.tile([NPOS, F], F32, tag=f"{tag}_sc2")
                nc.vector.tensor_mul(sc2[:], sc[:], r3[:])
                bc = sc2[:].unsqueeze(2).broadcast_to((NPOS, F, DO))
                nc.vector.tensor_mul(
                    dst_ap, s[:].rearrange("p (f o) -> p f o", o=DO), bc
                )

            # ---------------- stage B: out0 ----------------
            o0p = ps1.tile([NPOS, F * DO], F32, tag="ps1")
            for t in range(NCH):
                nc.tensor.matmul(
                    o0p[:],
                    p_ct[:, t * NPOS:(t + 1) * NPOS],
                    w_r[:, t * F * DO:(t + 1) * F * DO],
                    start=(t == 0),
                    stop=(t == NCH - 1),
                )
            out0_pad = work.tile([NPOS, F * 32], F32, tag="out0_pad")
            nc.vector.memset(out0_pad[:], 0.0)
            squash(
                o0p[:],
                out0_pad[:].rearrange("p (f s) -> p f s", s=32)[:, :, 0:DO],
                1.0 / F,
                "sq1",
            )
            # transposes -> out0T [128 = (j,o-slot), 8*72]
            out0T = work.tile([128, 8 * NPOS], F16, tag="out0T")
            for g in range(8):
                tp = ps1.tile([128, 128], F32, tag="ps1")
                nc.tensor.transpose(
                    tp[:, 0:NPOS], out0_pad[:, g * 128:(g + 1) * 128], eye72[:]
                )
                nc.scalar.copy(out0T[:, g * NPOS:(g + 1) * NPOS], tp[:, 0:NPOS])

            # ---------------- stage D: V, VP, agreement, exp ----------------
            # e[(c-rows), (f, 144 full | 72 packed)] = exp(agr)
            e = work.tile([128, F * ASL], F16, tag="e")
            TCH = NCH // 3      # 6 chunks per third
            TSZ = TCH * NPOS    # 432 cols = 1728 B = one fp32 PSUM bank
            def emit_agr(g, VPs):
                for jp in range(2):
                    # agr for a j-pair in one 1-bank psum tile; single exp.
                    # Per f one accumulation group covers both the full-chunk
                    # slice (identity weight, N=144) and the packed slice
                    # (block-ones weight -> 4-row-group replicated sum, N=72).
                    agp = ps1.tile([128, 2 * ASL], F32, tag="ps1",
                                   name=f"agp{g}_{jp}")
                    for jj in range(2):
                        j = 2 * jp + jj
                        base = jj * ASL
                        for i in range(DI):
                            nc.tensor.matmul(
                                agp[:, base:base + FSL],
                                eye128h[:],
                                VPs[j][:, i * FSL:(i + 1) * FSL],
                                start=(i == 0),
                                stop=False,
                            )
                        for ip in range(2):
                            nc.tensor.matmul(
                                agp[:, base + FSL:base + ASL],
                                bones[:],
                                VPs[j][:, NFULL * NPOS + ip * NPOS:
                                        NFULL * NPOS + (ip + 1) * NPOS],
                                start=False,
                                stop=(ip == 1),
                            )
                    f0 = 4 * g + 2 * jp
                    nc.scalar.activation(
                        e[:, f0 * ASL:(f0 + 2) * ASL], agp[:], AF.Exp
                    )

            prev = None
            for g in range(8):
                # 4 f's (one per row-band j) in flight: consecutive matmuls
                # target different 32-row groups so LDWEIGHTS can overlap.
                VPs = [vpr.tile([128, NCC], F16, tag="VP", name=f"VP{g}_{jj}")
                       for jj in range(4)]
                for q in range(3):
                    vqs = [vqp.tile([128, TSZ], F32, tag="vq",
                                    name=f"vq{g}_{q}_{jj}")
                           for jj in range(4)]
                    for u in range(TCH):
                        t = q * TCH + u
                        for j in range(4):
                            nc.tensor.matmul(
                                vqs[j][:, u * NPOS:(u + 1) * NPOS],
                                w_vt[32 * j:32 * (j + 1),
                                     (g * NCH + t) * 128:
                                     (g * NCH + t + 1) * 128],
                                out0T[32 * j:32 * (j + 1),
                                      g * NPOS:(g + 1) * NPOS],
                                start=True,
                                stop=True,
                                tile_position=(32 * j, 0),
                            )
                    for j in range(4):
                        if j == 0 or (j == 2 and q < 2):
                            # VP = V * P straight from PSUM on DVE (1x)
                            nc.vector.tensor_mul(
                                VPs[j][:, q * TSZ:(q + 1) * TSZ],
                                vqs[j][:],
                                p_ct[:, q * TSZ:(q + 1) * TSZ],
                            )
                        else:
                            # ScalarE evacuation -> fp16 SBUF, DVE 2x mult
                            vs = sr.tile([128, TSZ], F16, tag="vs")
                            nc.scalar.copy(vs[:], vqs[j][:])
                            nc.vector.tensor_mul(
                                VPs[j][:, q * TSZ:(q + 1) * TSZ],
                                vs[:],
                                p_ct[:, q * TSZ:(q + 1) * TSZ],
                            )
                if prev is not None:
                    emit_agr(g - 1, prev)
                prev = VPs
            emit_agr(7, prev)

            # ---------------- softmax pieces ----------------
            # Zf = sum_f e  (identity-matmul accumulation)
            Zf = ps1.tile([128, ASL], F32, tag="ps1")
            for f in range(F):
                nc.tensor.matmul(
                    Zf[:],
                    eye128h[:],
                    e[:, f * ASL:(f + 1) * ASL],
                    start=(f == 0),
                    stop=(f == F - 1),
                )
            Zr = work.tile([128, ASL], F16, tag="Zr")
            with nc.allow_low_precision(reason="1/Z in [1e-2,1e-1]; fp16 ok"):
                nc.vector.reciprocal(Zr[:], Zf[:])
            pp = work.tile([128, NCC], F16, tag="pp")
            nc.vector.tensor_mul(
                pp[:, 0:NFULL * NPOS]
                .rearrange("p (i cb n) -> p i cb n", i=DI, cb=2),
                p_ct[:, 0:NFULL * NPOS]
                .rearrange("p (i cb n) -> p i cb n", i=DI, cb=2),
                Zr[:, 0:FSL].rearrange("p (cb n) -> p cb n", cb=2)
                .unsqueeze(1)
                .broadcast_to((128, DI, 2, NPOS)),
            )
            nc.vector.tensor_mul(
                pp[:, NFULL * NPOS:NCC].rearrange("p (ip n) -> p ip n", ip=2),
                p_ct[:, NFULL * NPOS:NCC].rearrange("p (ip n) -> p ip n", ip=2),
                Zr[:, FSL:ASL].unsqueeze(1).broadcast_to((128, 2, NPOS)),
            )

            # ---------------- stage S + centroids ----------------
            cen_sb = work.tile([128, 8 * NPOS], F32, tag="cen_sb")
            for g in range(8):
                cg = ps1.tile([128, NPOS], F32, tag="ps1")
                for j in range(4):
                    f = 4 * g + j
                    S = sr.tile([128, NCC], F16, tag="S")
                    nc.vector.tensor_mul(
                        S[:, 0:NFULL * NPOS]
                        .rearrange("p (i cb n) -> p i cb n", i=DI, cb=2),
                        pp[:, 0:NFULL * NPOS]
                        .rearrange("p (i cb n) -> p i cb n", i=DI, cb=2),
                        e[:, f * ASL:f * ASL + FSL]
                        .rearrange("p (cb n) -> p cb n", cb=2)
                        .unsqueeze(1)
                        .broadcast_to((128, DI, 2, NPOS)),
                    )
                    nc.vector.tensor_mul(
                        S[:, NFULL * NPOS:NCC]
                        .rearrange("p (ip n) -> p ip n", ip=2),
                        pp[:, NFULL * NPOS:NCC]
                        .rearrange("p (ip n) -> p ip n", ip=2),
                        e[:, f * ASL + FSL:(f + 1) * ASL]
                        .unsqueeze(1).broadcast_to((128, 2, NPOS)),
                    )
                    for t in range(NCH):
                        nc.tensor.matmul(
                            cg[32 * j:32 * j + DO, :],
                            w_r[:, t * F * DO + f * DO:t * F * DO + (f + 1) * DO],
                            S[:, t * NPOS:(t + 1) * NPOS],
                            start=(t == 0),
                            stop=(t == NCH - 1),
                            tile_position=(0, 32 * j),
                        )
                nc.scalar.copy(cen_sb[:, g * NPOS:(g + 1) * NPOS], cg[:])

            # ---------------- squash2 + output ----------------
            opre = work.tile([NPOS, 8 * 128], F32, tag="opre")
            for g in range(8):
                tp2 = ps1.tile([128, 128], F32, tag="ps1")
                nc.tensor.transpose(
                    tp2[0:NPOS, :], cen_sb[:, g * NPOS:(g + 1) * NPOS], eye128f[:]
                )
                nc.scalar.copy(opre[:, g * 128:(g + 1) * 128], tp2[0:NPOS, :])
            gat = work.tile([NPOS, F * DO], F32, tag="gat")
            nc.vector.tensor_copy(
                gat[:].rearrange("p (g j o) -> p g j o", g=8, j=4),
                opre[:].rearrange("p (g j s) -> p g j s", g=8, j=4)[:, :, :, 0:DO],
            )
            y_sb = work.tile([NPOS, F * DO], F32, tag="y_sb")
            squash(gat[:], y_sb[:].rearrange("p (f o) -> p f o", o=DO), 1.0, "sq2")
            nc.sync.dma_start(y_d[:], y_sb[:])

    nc.compile()
    return nc


def _get_program():
    if "nc" not in _CACHE:
        _CACHE["nc"] = _build()
    return _CACHE["nc"]


def _make_in_maps(x, W):
    x = np.asarray(x, np.float32)
    Wm = np.asarray(W, np.float32)[0, 0, 0]
    w_r, w_vt = _host_weights(Wm)
    eye72 = np.eye(NPOS, dtype=np.float32)
    eye128h = np.eye(128, dtype=np.float16)
    bones = np.tile(np.eye(32, dtype=np.float16), (4, 4))
    eye128f = np.eye(128, dtype=np.float32)
    in_maps = []
    for k in range(8):
        in_maps.append({
            "p_ct": _host_patches(x, k),
            "w_r": w_r,
            "w_vt": w_vt,
            "eye72": eye72,
            "eye128h": eye128h,
            "bones": bones,
            "eye128f": eye128f,
        })
    return in_maps


def kernel(x, W):
    from concourse.bass_utils import run_bass_kernel_spmd

    nc = _get_program()
    in_maps = _make_in_maps(x, W)
    res = run_bass_kernel_spmd(nc, in_maps, list(range(8)))
    Ho, Wo = H - KH + 1, WD - KW + 1
    y = np.empty((B, Ho, Wo, F, DO), np.float32)
    for k in range(8):
        b, hh = divmod(k, 2)
        y[b, 6 * hh:6 * hh + 6] = res.results[k]["y"].reshape(6, Wo, F, DO)
    return y


# revision 19
# speedup vs baseline: 1.1317x; 1.0319x over previous
"""Trainium2 Bass kernel for the ConvolutionalCapsule module.

Sharding: data-parallel over (batch, H-half): core k handles b = k//2,
output rows h in [6*(k%2), 6*(k%2)+6), i.e. 72 spatial positions per core.
Weights are replicated. All FLOPs run on-device; the host only does
layout/gather of inputs (patch extraction + weight transposes).

Device algorithm per core (pos = 72 positions), with the (c,i) axis packed
densely into 18 chunks of 128 rows:
  - 16 "full" chunks: chunk u = i*2+cb holds c in [cb*128,(cb+1)*128) x {i}
  - 2 "packed" chunks: chunk 16+ip holds rows (isub*32 + c-256) for
    c in [256,288), i = 4*ip+isub  (the 32-wide tail c-block x 4 i's)

  iter0:  out0 = squash(mean_c preds) via K=(c,i)=2304 matmuls (stage B).
  iter1:  V[f,(c,i)] = sum_o W * out0   (per-f K=16 matmuls -> fp32 PSUM,
                                         fp8 weights, 4 row-bands in flight)
          VP = V * P                    (DVE direct from PSUM or ScalarE
                                         evacuation + DVE 2x mult)
          agr = sum_i VP                (identity matmuls N=144 for full
                                         chunks + block-ones matmuls N=72
                                         for packed chunks, PSUM accum)
          e = exp(agr) on ScalarE straight out of PSUM (one per f-pair),
          Z = sum_f e via identity-matmul accumulation,
          pp = P / Z, S_f = e_f * pp,
          centroids = sum_{(c,i)} S_f * W  (per-f K=128 chunk matmuls)
          out = squash(centroids)
The packed-chunk agr rows are replicated across the four 32-row groups
(block-ones weight), so exp/Z/pp/S stay purely elementwise.
"""
import numpy as np

KH = KW = 3
B, H, WD, FIN, DIN = 4, 14, 14, 32, 8
F, C, DO, DI = 32, 288, 16, 8
NPOS = 72
NCH = 18            # dense chunks
NFULL = 16
EPS = 1e-7

W8 = True   # fp8 weights for the V-stage (w_vt)

_CACHE: dict = {}


def _chunks_p():
    """Per chunk: (c_idx[128], i_idx[128]) — row r holds (c_idx[r], i_idx[r])."""
    chunks = []
    for i in range(DI):
        for cb in range(2):
            c = np.arange(cb * 128, (cb + 1) * 128)
            chunks.append((c, np.full(128, i)))
    r = np.arange(128)
    for ip in range(2):
        chunks.append((256 + r % 32, 4 * ip + r // 32))
    return chunks


def _host_weights(Wm):
    """Wm: [F, C, DO, DI] float32 -> device weight layouts."""
    chunks = _chunks_p()
    w_r = np.empty((NCH, 128, F * DO), np.float16)
    w_vt = np.zeros((4, 32, 8, NCH, 128), np.float32)
    for u, (c_idx, i_idx) in enumerate(chunks):
        # [128, F, DO] with row r = (c_idx[r], i_idx[r])
        blk = Wm[:, c_idx, :, i_idx]
        w_r[u] = blk.reshape(128, F * DO)
        for f in range(F):
            g, j = divmod(f, 4)
            w_vt[j, :DO, g, u, :] = blk[:, f, :].T
    w_r = w_r.transpose(1, 0, 2).reshape(128, NCH * F * DO).copy()
    w_vt = w_vt.reshape(128, 8 * NCH * 128)
    if W8:
        import ml_dtypes
        w_vt = w_vt.astype(ml_dtypes.float8_e4m3fn)
    else:
        w_vt = w_vt.astype(np.float16)
    return w_r, w_vt


def _host_patches(x, k):
    """Patch tensor for core k in dense chunk layout: [128, 18*72] fp16."""
    b, hh = divmod(k, 2)
    h0 = 6 * hh
    P = np.empty((6, 12, KH, KW, FIN, DIN), np.float32)
    for kh in range(KH):
        for kw in range(KW):
            for h in range(6):
                P[h, :, kh, kw] = x[b, h0 + h + kh, kw:kw + 12]
    P = P.reshape(NPOS, C, DIN)
    p_ct = np.empty((NCH, 128, NPOS), np.float16)
    for u, (c_idx, i_idx) in enumerate(_chunks_p()):
        p_ct[u] = P[:, c_idx, i_idx].T
    return p_ct.transpose(1, 0, 2).reshape(128, NCH * NPOS).copy()


def _build():
    import concourse.bass as bass
    import concourse.bacc as bacc
    import concourse.mybir as mybir
    import concourse.tile as tile

    F16, F32 = mybir.dt.float16, mybir.dt.float32
    W_DT = mybir.dt.float8e4 if W8 else F16
    AX = mybir.AxisListType
    AF = mybir.ActivationFunctionType

    nc = bacc.Bacc(None, target_bir_lowering=False, debug=False)

    p_ct_d = nc.dram_tensor("p_ct", [128, NCH * NPOS], F16, kind="ExternalInput")
    w_r_d = nc.dram_tensor("w_r", [128, NCH * F * DO], F16, kind="ExternalInput")
    w_vt_d = nc.dram_tensor("w_vt", [128, 8 * NCH * 128], W_DT, kind="ExternalInput")
    eye72_d = nc.dram_tensor("eye72", [NPOS, NPOS], F32, kind="ExternalInput")
    eye128h_d = nc.dram_tensor("eye128h", [128, 128], F16, kind="ExternalInput")
    bones_d = nc.dram_tensor("bones", [128, 128], F16, kind="ExternalInput")
    eye128f_d = nc.dram_tensor("eye128f", [128, 128], F32, kind="ExternalInput")
    y_d = nc.dram_tensor("y", [NPOS, F * DO], F32, kind="ExternalOutput")

    NCC = NCH * NPOS          # 1296 cols of P/VP/S
    FSL = 2 * NPOS            # 144 = full-chunk slice of agr (cb,pos)
    PSL = NPOS                # 72  = packed-chunk (replicated) slice
    ASL = FSL + PSL           # 216 = per-f agr/e slice

    with tile.TileContext(nc) as tc:
        with (
            tc.tile_pool(name="const", bufs=1) as const,
            tc.tile_pool(name="work", bufs=1) as work,
            tc.tile_pool(name="vpr", bufs=6) as vpr,
            tc.tile_pool(name="sr", bufs=6) as sr,
            tc.tile_pool(name="vqp", bufs=5, space=bass.MemorySpace.PSUM) as vqp,
            tc.tile_pool(name="ps1", bufs=3, space=bass.MemorySpace.PSUM) as ps1,
        ):
            # ---------------- loads ----------------
            p_ct = const.tile([128, NCC], F16, tag="p_ct")
            nc.sync.dma_start(p_ct[:], p_ct_d[:])
            w_r = const.tile([128, NCH * F * DO], F16, tag="w_r")
            WRC = F * DO
            for h in range(3):
                nc.sync.dma_start(
                    w_r[:, h * 6 * WRC:(h + 1) * 6 * WRC],
                    w_r_d[:, h * 6 * WRC:(h + 1) * 6 * WRC],
                )
            eye72 = const.tile([NPOS, NPOS], F32, tag="eye72")
            nc.sync.dma_start(eye72[:], eye72_d[:])
            eye128h = const.tile([128, 128], F16, tag="eye128h")
            nc.sync.dma_start(eye128h[:], eye128h_d[:])
            bones = const.tile([128, 128], F16, tag="bones")
            nc.sync.dma_start(bones[:], bones_d[:])
            eye128f = const.tile([128, 128], F32, tag="eye128f")
            nc.sync.dma_start(eye128f[:], eye128f_d[:])
            w_vt = const.tile([128, 8 * NCH * 128], W_DT, tag="w_vt")
            GSZ = NCH * 128
            for g in range(8):
                nc.sync.dma_start(
                    w_vt[:, g * GSZ:(g + 1) * GSZ],
                    w_vt_d[:, g * GSZ:(g + 1) * GSZ],
                )

            def squash(src_ap, dst_ap, pre_scale, tag, nf=F):
                """dst = squash(src * pre_scale) ; src/dst free = (nf,o)."""
                s = work.tile([NPOS, nf * DO], F32, tag=f"{tag}_s")
                nc.scalar.activation(s[:], src_ap, AF.Copy, scale=pre_scale)
                sq = work.tile([NPOS, nf * DO], F32, tag=f"{tag}_sq")
                nc.scalar.activation(sq[:], s[:], AF.Square)
                sn = work.tile([NPOS, nf], F32, tag=f"{tag}_sn")
                nc.vector.reduce_sum(
                    sn[:], sq[:].rearrange("p (f o) -> p f o", o=DO), axis=AX.X
                )
                t1 = work.tile([NPOS, nf], F32, tag=f"{tag}_t1")
                nc.vector.tensor_scalar_add(t1[:], sn[:], 1.0)
                r1 = work.tile([NPOS, nf], F32, tag=f"{tag}_r1")
                nc.vector.reciprocal(r1[:], t1[:])
                se = work.tile([NPOS, nf], F32, tag=f"{tag}_se")
                nc.vector.tensor_scalar_add(se[:], sn[:], EPS)
                r2 = work.tile([NPOS, nf], F32, tag=f"{tag}_r2")
                nc.scalar.activation(r2[:], se[:], AF.Sqrt)
                r3 = work.tile([NPOS, nf], F32, tag=f"{tag}_r3")
                nc.vector.reciprocal(r3[:], r2[:])
                sc = work.tile([NPOS, nf], F32, tag=f"{tag}_sc")
                nc.vector.tensor_mul(sc[:], sn[:], r1[:])
                sc2 = work.tile([NPOS, nf], F32, tag=f"{tag}_sc2")
                nc.vector.tensor_mul(sc2[:], sc[:], r3[:])
                bc = sc2[:].unsqueeze(2).broadcast_to((NPOS, nf, DO))
                nc.vector.tensor_mul(
                    dst_ap, s[:].rearrange("p (f o) -> p f o", o=DO), bc
                )

            # ---------------- stage B: out0 ----------------
            o0p = ps1.tile([NPOS, F * DO], F32, tag="ps1")
            for t in range(NCH):
                nc.tensor.matmul(
                    o0p[:],
                    p_ct[:, t * NPOS:(t + 1) * NPOS],
                    w_r[:, t * F * DO:(t + 1) * F * DO],
                    start=(t == 0),
                    stop=(t == NCH - 1),
                )
            out0_pad = work.tile([NPOS, F * 32], F32, tag="out0_pad")
            nc.vector.memset(out0_pad[:], 0.0)
            for hh in range(2):
                squash(
                    o0p[:, hh * 256:(hh + 1) * 256],
                    out0_pad[:, hh * 512:(hh + 1) * 512]
                    .rearrange("p (f s) -> p f s", s=32)[:, :, 0:DO],
                    1.0 / F,
                    f"sq1_{hh}",
                    nf=16,
                )
            # transposes -> out0T [128 = (j,o-slot), 8*72]
            out0T = work.tile([128, 8 * NPOS], F16, tag="out0T")
            for g in range(8):
                tp = ps1.tile([128, 128], F32, tag="ps1")
                nc.tensor.transpose(
                    tp[:, 0:NPOS], out0_pad[:, g * 128:(g + 1) * 128], eye72[:]
                )
                nc.scalar.copy(out0T[:, g * NPOS:(g + 1) * NPOS], tp[:, 0:NPOS])

            # ---------------- stage D: V, VP, agreement, exp ----------------
            # e[(c-rows), (f, 144 full | 72 packed)] = exp(agr)
            e = work.tile([128, F * ASL], F16, tag="e")
            TCH = NCH // 3      # 6 chunks per third
            TSZ = TCH * NPOS    # 432 cols = 1728 B = one fp32 PSUM bank
            Zf = ps1.tile([128, ASL], F32, tag="ps1")

            def emit_agr(g, VPs):
                for jp in range(2):
                    # agr for a j-pair in one 1-bank psum tile; single exp.
                    # Per f one accumulation group covers both the full-chunk
                    # slice (identity weight, N=144) and the packed slice
                    # (block-ones weight -> 4-row-group replicated sum, N=72).
                    agp = ps1.tile([128, 2 * ASL], F32, tag="ps1",
                                   name=f"agp{g}_{jp}")
                    for jj in range(2):
                        j = 2 * jp + jj
                        base = jj * ASL
                        for i in range(DI):
                            nc.tensor.matmul(
                                agp[:, base:base + FSL],
                                eye128h[:],
                                VPs[j][:, i * FSL:(i + 1) * FSL],
                                start=(i == 0),
                                stop=False,
                            )
                        for ip in range(2):
                            nc.tensor.matmul(
                                agp[:, base + FSL:base + ASL],
                                bones[:],
                                VPs[j][:, NFULL * NPOS + ip * NPOS:
                                        NFULL * NPOS + (ip + 1) * NPOS],
                                start=False,
                                stop=(ip == 1),
                            )
                    f0 = 4 * g + 2 * jp
                    nc.scalar.activation(
                        e[:, f0 * ASL:(f0 + 2) * ASL], agp[:], AF.Exp
                    )
                    for f in (f0, f0 + 1):
                        nc.tensor.matmul(
                            Zf[:],
                            eye128h[:],
                            e[:, f * ASL:(f + 1) * ASL],
                            start=(f == 0),
                            stop=(f == F - 1),
                        )

            prev = None
            for g in range(8):
                # 4 f's (one per row-band j) in flight: consecutive matmuls
                # target different 32-row groups so LDWEIGHTS can overlap.
                VPs = [vpr.tile([128, NCC], F16, tag="VP", name=f"VP{g}_{jj}")
                       for jj in range(4)]
                for q in range(3):
                    vqs = [vqp.tile([128, TSZ], F32, tag="vq",
                                    name=f"vq{g}_{q}_{jj}")
                           for jj in range(4)]
                    for u in range(TCH):
                        t = q * TCH + u
                        for j in range(4):
                            nc.tensor.matmul(
                                vqs[j][:, u * NPOS:(u + 1) * NPOS],
                                w_vt[32 * j:32 * (j + 1),
                                     (g * NCH + t) * 128:
                                     (g * NCH + t + 1) * 128],
                                out0T[32 * j:32 * (j + 1),
                                      g * NPOS:(g + 1) * NPOS],
                                start=True,
                                stop=True,
                                tile_position=(32 * j, 0),
                            )
                    for j in range(4):
                        if j == 0 or (j == 2 and q < 2):
                            # VP = V * P straight from PSUM on DVE (1x)
                            nc.vector.tensor_mul(
                                VPs[j][:, q * TSZ:(q + 1) * TSZ],
                                vqs[j][:],
                                p_ct[:, q * TSZ:(q + 1) * TSZ],
                            )
                        else:
                            # ScalarE evacuation -> fp16 SBUF, DVE 2x mult
                            vs = sr.tile([128, TSZ], F16, tag="vs")
                            nc.scalar.copy(vs[:], vqs[j][:])
                            nc.vector.tensor_mul(
                                VPs[j][:, q * TSZ:(q + 1) * TSZ],
                                vs[:],
                                p_ct[:, q * TSZ:(q + 1) * TSZ],
                            )
                if prev is not None:
                    emit_agr(g - 1, prev)
                prev = VPs
            emit_agr(7, prev)

            # ---------------- softmax pieces ----------------
            Zr = work.tile([128, ASL], F16, tag="Zr")
            with nc.allow_low_precision(reason="1/Z in [1e-2,1e-1]; fp16 ok"):
                nc.vector.reciprocal(Zr[:], Zf[:])
            pp = work.tile([128, NCC], F16, tag="pp")
            nc.vector.tensor_mul(
                pp[:, 0:NFULL * NPOS]
                .rearrange("p (i cb n) -> p i cb n", i=DI, cb=2),
                p_ct[:, 0:NFULL * NPOS]
                .rearrange("p (i cb n) -> p i cb n", i=DI, cb=2),
                Zr[:, 0:FSL].rearrange("p (cb n) -> p cb n", cb=2)
                .unsqueeze(1)
                .broadcast_to((128, DI, 2, NPOS)),
            )
            nc.vector.tensor_mul(
                pp[:, NFULL * NPOS:NCC].rearrange("p (ip n) -> p ip n", ip=2),
                p_ct[:, NFULL * NPOS:NCC].rearrange("p (ip n) -> p ip n", ip=2),
                Zr[:, FSL:ASL].unsqueeze(1).broadcast_to((128, 2, NPOS)),
            )

            # ---------------- stage S + centroids ----------------
            cen_sb = work.tile([128, 8 * NPOS], F32, tag="cen_sb")
            for g in range(8):
                cg = ps1.tile([128, NPOS], F32, tag="ps1")
                for j in range(4):
                    f = 4 * g + j
                    S = sr.tile([128, NCC], F16, tag="S")
                    nc.vector.tensor_mul(
                        S[:, 0:NFULL * NPOS]
                        .rearrange("p (i cb n) -> p i cb n", i=DI, cb=2),
                        pp[:, 0:NFULL * NPOS]
                        .rearrange("p (i cb n) -> p i cb n", i=DI, cb=2),
                        e[:, f * ASL:f * ASL + FSL]
                        .rearrange("p (cb n) -> p cb n", cb=2)
                        .unsqueeze(1)
                        .broadcast_to((128, DI, 2, NPOS)),
                    )
                    nc.vector.tensor_mul(
                        S[:, NFULL * NPOS:NCC]
                        .rearrange("p (ip n) -> p ip n", ip=2),
                        pp[:, NFULL * NPOS:NCC]
                        .rearrange("p (ip n) -> p ip n", ip=2),
                        e[:, f * ASL + FSL:(f + 1) * ASL]
                        .unsqueeze(1).broadcast_to((128, 2, NPOS)),
                    )
                    for t in range(NCH):
                        nc.tensor.matmul(
                            cg[32 * j:32 * j + DO, :],
                            w_r[:, t * F * DO + f * DO:t * F * DO + (f + 1) * DO],
                            S[:, t * NPOS:(t + 1) * NPOS],
                            start=(t == 0),
                            stop=(t == NCH - 1),
                            tile_position=(0, 32 * j),
                        )
                nc.scalar.copy(cen_sb[:, g * NPOS:(g + 1) * NPOS], cg[:])

            # ---------------- squash2 + output ----------------
            opre = work.tile([NPOS, 8 * 128], F32, tag="opre")
            for g in range(8):
                tp2 = ps1.tile([128, 128], F32, tag="ps1")
                nc.tensor.transpose(
                    tp2[0:NPOS, :], cen_sb[:, g * NPOS:(g + 1) * NPOS], eye128f[:]
                )
                nc.scalar.copy(opre[:, g * 128:(g + 1) * 128], tp2[0:NPOS, :])
            gat = work.tile([NPOS, F * DO], F32, tag="gat")
            nc.vector.tensor_copy(
                gat[:].rearrange("p (g j o) -> p g j o", g=8, j=4),
                opre[:].rearrange("p (g j s) -> p g j s", g=8, j=4)[:, :, :, 0:DO],
            )
            y_sb = work.tile([NPOS, F * DO], F32, tag="y_sb")
            squash(gat[:], y_sb[:].rearrange("p (f o) -> p f o", o=DO), 1.0, "sq2")
            nc.sync.dma_start(y_d[:], y_sb[:])

    nc.compile()
    return nc


def _get_program():
    if "nc" not in _CACHE:
        _CACHE["nc"] = _build()
    return _CACHE["nc"]


def _make_in_maps(x, W):
    x = np.asarray(x, np.float32)
    Wm = np.asarray(W, np.float32)[0, 0, 0]
    w_r, w_vt = _host_weights(Wm)
    eye72 = np.eye(NPOS, dtype=np.float32)
    eye128h = np.eye(128, dtype=np.float16)
    bones = np.tile(np.eye(32, dtype=np.float16), (4, 4))
    eye128f = np.eye(128, dtype=np.float32)
    in_maps = []
    for k in range(8):
        in_maps.append({
            "p_ct": _host_patches(x, k),
            "w_r": w_r,
            "w_vt": w_vt,
            "eye72": eye72,
            "eye128h": eye128h,
            "bones": bones,
            "eye128f": eye128f,
        })
    return in_maps


def kernel(x, W):
    from concourse.bass_utils import run_bass_kernel_spmd

    nc = _get_program()
    in_maps = _make_in_maps(x, W)
    res = run_bass_kernel_spmd(nc, in_maps, list(range(8)))
    Ho, Wo = H - KH + 1, WD - KW + 1
    y = np.empty((B, Ho, Wo, F, DO), np.float32)
    for k in range(8):
        b, hh = divmod(k, 2)
        y[b, 6 * hh:6 * hh + 6] = res.results[k]["y"].reshape(6, Wo, F, DO)
    return y
